# revision 1
# baseline (speedup 1.0000x reference)
"""Trainium2 Bass kernel for LocalPatternFilter.

Reference computation (per (b, h, c) row of length K=1024):
  1. gather window  g = X[b, c, pos[h] : pos[h]+K]
  2. fmax = max|g|;  w = g * hann / fmax
  3. acov = ifftshift(irfft(|rfft(w)|^2))   (= circular autocorrelation)

Implemented as dense DFT matmuls on the tensor engine (float32r, 1 col/cyc):
  - rfft via real matmuls with cos/sin matrices (hann window folded into the
    matrices; 9 m-tiles of 128: cos m=0..639 with m>512 zeroed, sin m=0..511)
  - power spectrum P = Re^2 + Im^2 (P[m>512] = 0 automatically)
  - irfft + ifftshift + output symmetry folded into one (640 x 1024) cos
    matrix C2:  y[j] = acov[(j+512) mod 1024], using acov[n] = acov[K-n].
  - 1/fmax^2 folded into the final PSUM->SBUF copy (acov is linear in P).

Sharding: data-parallel over batch, 2 batches per core on 8 cores.
"""

import json

import numpy as np

import concourse.bass as bass
import concourse.bass2jax as bass2jax
import concourse.bass_utils as bass_utils
import concourse.tile as tile
from concourse import mybir
from concourse.bass_utils import run_bass_kernel_spmd

# ---------------------------------------------------------------------------
# The walrus build in this container accepts at most ONE sync-wait command per
# TPB instruction ("Too many sync wait commands" in setupSyncWait), while Tile
# emits several (multi-queue DMA deps, the kernel-tail drain). Legalize the
# serialized BIR before compiling: hoist excess waits onto preceding
# same-engine wait-only EventSemaphore instructions. Engines execute their
# instruction streams in order, so this is semantically identical.
# ---------------------------------------------------------------------------
_MAX_WAITS = 1


def _legalize_bir(bir_bytes):
    m = json.loads(bir_bytes)
    counter = [0]

    def fix_block(blk):
        insts = blk.get("instructions")
        if not isinstance(insts, list):
            return
        out = []
        for inst in insts:
            si = inst.get("sync_info") or {}
            waits = si.get("on_wait") or []
            if isinstance(inst.get("opcode"), str) and len(waits) > _MAX_WAITS:
                keep = waits[-_MAX_WAITS:]
                for w in waits[:-_MAX_WAITS]:
                    counter[0] += 1
                    out.append(
                        {
                            "debug": inst.get("debug", 0),
                            "engine": inst["engine"],
                            "ins": [],
                            "name": f"LGW-{counter[0]}-{inst['name']}",
                            "opcode": "EventSemaphore",
                            "outs": [],
                            "sync_info": {"on_update": [], "on_wait": [w]},
                        }
                    )
                si = dict(si)
                si["on_wait"] = keep
                inst = dict(inst)
                inst["sync_info"] = si
            out.append(inst)
        blk["instructions"] = out

    def walk(o):
        if isinstance(o, dict):
            if "instructions" in o:
                fix_block(o)
            for v in o.values():
                walk(v)
        elif isinstance(o, list):
            for v in o:
                walk(v)

    walk(m)
    return json.dumps(m).encode()


_orig_compile_bir_kernel = bass_utils.compile_bir_kernel


def _legalizing_compile_bir_kernel(bir_json, *args, **kwargs):
    if isinstance(bir_json, str):
        bir_json = bir_json.encode()
    return _orig_compile_bir_kernel(_legalize_bir(bir_json), *args, **kwargs)


if bass_utils.compile_bir_kernel is not _legalizing_compile_bir_kernel:
    bass_utils.compile_bir_kernel = _legalizing_compile_bir_kernel
    bass2jax.compile_bir_kernel = _legalizing_compile_bir_kernel

# positions = int32(jnp.linspace(0, L-2K-1, H)) for L=131072, K=1024, H=128
# (verified identical to the jax reference's values)
POS = [
    0, 1015, 2031, 3047, 4063, 5079, 6095, 7111, 8127, 9143, 10159, 11175,
    12191, 13207, 14223, 15238, 16254, 17270, 18286, 19302, 20318, 21334,
    22350, 23366, 24382, 25398, 26414, 27430, 28446, 29461, 30477, 31493,
    32509, 33525, 34541, 35557, 36573, 37589, 38605, 39621, 40637, 41653,
    42669, 43684, 44700, 45716, 46732, 47748, 48764, 49780, 50796, 51812,
    52828, 53844, 54860, 55876, 56892, 57907, 58923, 59939, 60955, 61971,
    62987, 64003, 65019, 66035, 67051, 68067, 69083, 70099, 71115, 72130,
    73146, 74162, 75178, 76194, 77210, 78226, 79242, 80258, 81274, 82290,
    83306, 84322, 85338, 86353, 87369, 88385, 89401, 90417, 91433, 92449,
    93465, 94481, 95497, 96513, 97529, 98545, 99561, 100576, 101592, 102608,
    103624, 104640, 105656, 106672, 107688, 108704, 109720, 110736, 111752,
    112768, 113784, 114799, 115815, 116831, 117847, 118863, 119879, 120895,
    121911, 122927, 123943, 124959, 125975, 126991, 128007, 129023,
]

N_CORES = 8
B_FULL, C_DIM, L_DIM = 16, 8, 131072
K_DIM, H_DIM = 1024, 128
B_LOC = B_FULL // N_CORES          # batches per core
M_PAD = 640                        # rfft bins 0..512, padded to 5*128
CS_W = 1152                        # 640 windowed-cos cols + 512 windowed-sin
F32 = mybir.dt.float32
F32R = mybir.dt.float32r

_prog_cache = {}


def make_constants():
    K = K_DIM
    k = np.arange(K)[:, None].astype(np.float64)
    hann = 0.5 * (1.0 - np.cos(2.0 * np.pi * np.arange(K) / K))[:, None]
    m = np.arange(M_PAD)[None, :].astype(np.float64)
    cs_cos = hann * np.cos(2 * np.pi * k * m / K)
    cs_cos[:, 513:] = 0.0
    m2 = np.arange(512)[None, :].astype(np.float64)
    cs_sin = hann * np.sin(2 * np.pi * k * m2 / K)
    # cs1[t, p, :] = [win*cos m=0..639 | win*sin m=0..511] for k = 128 t + p
    cs1 = np.concatenate([cs_cos, cs_sin], axis=1)  # (1024, 1152)
    cs1 = cs1.reshape(8, 128, CS_W).astype(np.float32)

    # irfft weights for acov[d], d = 0..512 only; the full output row is
    # y[512+d] = acov[d], y[j] = acov[512-j] (acov is even), written via a
    # reversed-stride copy.
    wt = np.ones(M_PAD)
    wt[1:512] = 2.0
    wt[512] = 1.0
    wt[513:] = 0.0
    mm = np.arange(M_PAD)[:, None].astype(np.float64)
    d = np.arange(513)[None, :].astype(np.float64)
    c2 = wt[:, None] * np.cos(2 * np.pi * mm * d / K) / K
    c2 = np.concatenate([c2, np.zeros((M_PAD, 7))], axis=1)  # pad: fp32r
    c2 = c2.reshape(5, 128, 520).astype(np.float32)          # matmul needs N>1

    ident = np.eye(128, dtype=np.float32)

    # gather start offsets (elements into flat [B_LOC*C*L] x) per tile row:
    # tile j = g*4 + i; row p = hh*8 + c; h = (g%2)*64 + 16*i + hh
    gidx = np.zeros((16, 128), dtype=np.int32)
    for g in range(2 * B_LOC):
        b = g // 2
        h0 = (g % 2) * 64
        for i in range(4):
            for hh in range(16):
                h = h0 + 16 * i + hh
                for c in range(C_DIM):
                    gidx[g * 4 + i, hh * 8 + c] = (
                        b * C_DIM * L_DIM + c * L_DIM + POS[h]
                    )
    return {"cs1": cs1, "c2": c2, "ident": ident, "gidx": gidx}


def build_program():
    nc = bass.Bass("TRN2", target_bir_lowering=False, debug=False,
                   num_swdge_queues=4)
    x = nc.dram_tensor("x", [B_LOC, C_DIM, L_DIM], F32, kind="ExternalInput").ap()
    cs1 = nc.dram_tensor("cs1", [8, 128, CS_W], F32R, kind="ExternalInput").ap()
    c2 = nc.dram_tensor("c2", [5, 128, 520], F32R, kind="ExternalInput").ap()
    ident = nc.dram_tensor("ident", [128, 128], F32, kind="ExternalInput").ap()
    gidx = nc.dram_tensor("gidx", [16, 128], mybir.dt.int32, kind="ExternalInput").ap()
    y = nc.dram_tensor(
        "y", [B_LOC, H_DIM, C_DIM, K_DIM], F32, kind="ExternalOutput"
    ).ap()

    with tile.TileContext(nc) as tc:
        with (
            tc.tile_pool(name="singles", bufs=1) as singles,
            tc.tile_pool(name="gather", bufs=8) as gpool,
            tc.tile_pool(name="wt", bufs=2) as wtpool,
            tc.tile_pool(name="pp", bufs=2) as ppool,
            tc.tile_pool(name="yy", bufs=4) as ypool,
            tc.tile_pool(name="small", bufs=16) as smallpool,
            tc.tile_pool(name="sq", bufs=3) as sqpool,
            tc.tile_pool(name="tp_ps", bufs=2, space="PSUM") as tp_ps_pool,
            tc.tile_pool(name="mm1_ps", bufs=2, space="PSUM") as mm1_ps_pool,
            tc.tile_pool(name="mm2_ps", bufs=1, space="PSUM") as mm2_ps_pool,
        ):
            # small control inputs first so gathers + transposes start
            # immediately; big DFT matrices stream in per-chunk behind them
            gidx_sb = singles.tile([128, 16], mybir.dt.int32)
            nc.sync.dma_start(out=gidx_sb, in_=gidx.rearrange("t p -> p t"))
            id_sb = singles.tile([128, 128], F32)
            nc.sync.dma_start(out=id_sb, in_=ident)
            x_flat = x.rearrange("b c l -> (b c) l")
            cs1_sb = singles.tile([128, 8, CS_W], F32R)
            cs1_r = cs1.rearrange("t p m -> p t m")
            for t in range(8):
                nc.sync.dma_start(out=cs1_sb[:, t, :], in_=cs1_r[:, t, :])
            c2_sb = singles.tile([128, 5, 520], F32R)
            c2_r = c2.rearrange("t p n -> p t n")
            for t in range(5):
                nc.sync.dma_start(out=c2_sb[:, t, :], in_=c2_r[:, t, :])

            # 4 groups of 512 rows; row = b*1024 + h*8 + c
            for g in range(2 * B_LOC):
                b = g // 2
                h0 = (g % 2) * 64
                gts = []
                inv2s = []
                for i in range(4):
                    gt = gpool.tile([128, K_DIM], F32, tag="gt")
                    j = g * 4 + i
                    gd = nc.gpsimd.indirect_dma_start(
                        out=gt[:],
                        out_offset=None,
                        in_=x_flat,
                        in_offset=bass.IndirectOffsetOnAxis(
                            ap=gidx_sb[:, j : j + 1], axis=1
                        ),
                    )
                    qi = j % 4
                    if qi:
                        gd.ins.queue = f"qPoolDynamic{qi}"  # spread SWDGE queues
                    fm = smallpool.tile([128, 1], F32, tag="fm")
                    nc.vector.reduce_max(
                        out=fm, in_=gt,
                        axis=mybir.AxisListType.X,
                        apply_absolute_value=True,
                    )
                    inv = smallpool.tile([128, 1], F32, tag="inv")
                    nc.vector.reciprocal(out=inv, in_=fm)
                    inv2 = smallpool.tile([128, 1], F32, tag="inv2")
                    nc.vector.tensor_mul(inv2, inv, inv)
                    gts.append(gt)
                    inv2s.append(inv2)

                # transpose gathered rows to [k, row] layout
                wt_t = wtpool.tile([128, 8, 512], F32R, tag="wt")
                for t in range(8):
                    tp = tp_ps_pool.tile([128, 512], F32, tag="tp")
                    for i in range(4):
                        nc.tensor.transpose(
                            tp[:, 128 * i : 128 * (i + 1)],
                            gts[i][:, 128 * t : 128 * (t + 1)],
                            id_sb,
                        )
                    nc.scalar.copy(out=wt_t[:, t, :], in_=tp)

                # mm1: spectrum tiles [m(128), row(512)]; P = Re^2 + Im^2
                p_sb = ppool.tile([128, 5, 512], F32R, tag="p")
                for pair in range(5):
                    mm1 = mm1_ps_pool.tile([128, 1024], F32, tag="mm1")
                    for kc in range(8):
                        nc.tensor.matmul(
                            mm1[:, 0:512],
                            cs1_sb[:, kc, 128 * pair : 128 * (pair + 1)],
                            wt_t[:, kc, :],
                            start=(kc == 0),
                            stop=(kc == 7),
                        )
                    if pair < 4:
                        for kc in range(8):
                            nc.tensor.matmul(
                                mm1[:, 512:1024],
                                cs1_sb[
                                    :, kc, 640 + 128 * pair : 640 + 128 * (pair + 1)
                                ],
                                wt_t[:, kc, :],
                                start=(kc == 0),
                                stop=(kc == 7),
                            )
                        sq = sqpool.tile([128, 512], F32, tag="sq")
                        nc.scalar.square(sq, mm1[:, 0:512])
                        tb = sqpool.tile([128, 512], F32, tag="tb")
                        nc.scalar.square(tb, mm1[:, 512:1024])
                        nc.vector.tensor_add(p_sb[:, pair, :], sq, tb)
                    else:
                        # sin(m) = 0 for the whole pad tile: P = cos^2 only
                        nc.scalar.square(p_sb[:, pair, :], mm1[:, 0:512])

                # mm2: acov[row, 0:513] = P.T @ C2, then expand by symmetry:
                # y[512+d] = acov[d], y[j] = acov[512-j]; scale by 1/fmax^2
                for rt in range(4):
                    mm2 = mm2_ps_pool.tile([128, 520], F32, tag="mm2")
                    for chunk in range(5):
                        nc.tensor.matmul(
                            mm2[:, 0:512],
                            p_sb[:, chunk, 128 * rt : 128 * (rt + 1)],
                            c2_sb[:, chunk, 0:512],
                            start=(chunk == 0),
                            stop=(chunk == 4),
                        )
                    for chunk in range(5):
                        nc.tensor.matmul(
                            mm2[:, 512:520],
                            p_sb[:, chunk, 128 * rt : 128 * (rt + 1)],
                            c2_sb[:, chunk, 512:520],
                            start=(chunk == 0),
                            stop=(chunk == 4),
                        )
                    ysb = ypool.tile([128, K_DIM], F32, tag="y")
                    nc.vector.tensor_scalar_mul(
                        ysb[:, 512:1024], mm2[:, 0:512], inv2s[rt]
                    )
                    rev = bass.AP(
                        tensor=mm2.tensor,
                        offset=mm2.offset + 512,
                        ap=[list(mm2.ap[0]), [-1, 512]],
                    )
                    nc.vector.tensor_scalar_mul(ysb[:, 0:512], rev, inv2s[rt])
                    hs = h0 + 16 * rt
                    dst = y[b, hs : hs + 16].rearrange("h c n -> (h c) n")
                    nc.sync.dma_start(out=dst, in_=ysb)
    return nc


def get_program():
    if "nc" not in _prog_cache:
        _prog_cache["nc"] = build_program()
        _prog_cache["consts"] = make_constants()
    return _prog_cache["nc"], _prog_cache["consts"]


def kernel(X, kernel_size=None, out_channels=None, _trace=False):
    X = np.ascontiguousarray(np.asarray(X, dtype=np.float32))
    assert X.shape == (B_FULL, C_DIM, L_DIM)
    nc, consts = get_program()
    in_maps = []
    for c in range(N_CORES):
        m = {"x": X[c * B_LOC : (c + 1) * B_LOC]}
        m.update(consts)
        in_maps.append(m)
    res = run_bass_kernel_spmd(
        nc, in_maps, core_ids=list(range(N_CORES)), trace=_trace
    )
    out = np.concatenate([r["y"] for r in res.results], axis=0)
    if _trace:
        return out, res
    return out



# revision 9
# speedup vs baseline: 1.1095x; 1.1095x over previous
"""Trainium2 Bass kernel for LocalPatternFilter.

Reference computation (per (b, h, c) row of length K=1024):
  1. gather window  g = X[b, c, pos[h] : pos[h]+K]
  2. fmax = max|g|;  w = g * hann / fmax
  3. acov = ifftshift(irfft(|rfft(w)|^2))   (= circular autocorrelation)

Implemented as dense DFT matmuls on the tensor engine in fp8e4(m3) with
MatmulPerfMode.DoubleRow (0.5 cycles/row, 2x bf16 throughput):
  - packed 1024-bin spectrum layout: cols [c0..c512, s1..s511] (513 cos +
    511 sin = exactly 1024 bins, hann folded into the matrix, data scaled
    by 1/8 during the PSUM->SBUF cast so P = F^2/64 fits fp8 range)
  - P[j] = F[j]^2 elementwise for ALL packed bins (no pair-add needed:
    the irfft matrix c2 carries the cos^2/sin^2 split, x64 to undo 1/64)
  - irfft + ifftshift + output symmetry folded into c2 (1024 x 513),
    out d=0..512; the other half written via a reversed-stride copy.
  - 1/fmax^2 folded into the final PSUM->SBUF copy (acov is linear in P);
    fmax computed in f32 from the raw gathered data.
  - output stored bf16, converted to f32 on host.

Sharding: data-parallel over batch, 2 batches per core on 8 cores.
"""

import json

import numpy as np
import ml_dtypes

import concourse.bass as bass
import concourse.bass2jax as bass2jax
import concourse.bass_utils as bass_utils
import concourse.tile as tile
from concourse import mybir
from concourse.bass_utils import run_bass_kernel_spmd

# ---------------------------------------------------------------------------
# The walrus build in this container accepts at most ONE sync-wait command per
# TPB instruction ("Too many sync wait commands" in setupSyncWait), while Tile
# emits several (multi-queue DMA deps, the kernel-tail drain). Legalize the
# serialized BIR before compiling: hoist excess waits onto preceding
# same-engine wait-only EventSemaphore instructions. Engines execute their
# instruction streams in order, so this is semantically identical.
# ---------------------------------------------------------------------------
_MAX_WAITS = 1


def _legalize_bir(bir_bytes):
    m = json.loads(bir_bytes)
    counter = [0]

    def fix_block(blk):
        insts = blk.get("instructions")
        if not isinstance(insts, list):
            return
        out = []
        for inst in insts:
            si = inst.get("sync_info") or {}
            waits = si.get("on_wait") or []
            if isinstance(inst.get("opcode"), str) and len(waits) > _MAX_WAITS:
                keep = waits[-_MAX_WAITS:]
                for w in waits[:-_MAX_WAITS]:
                    counter[0] += 1
                    out.append(
                        {
                            "debug": inst.get("debug", 0),
                            "engine": inst["engine"],
                            "ins": [],
                            "name": f"LGW-{counter[0]}-{inst['name']}",
                            "opcode": "EventSemaphore",
                            "outs": [],
                            "sync_info": {"on_update": [], "on_wait": [w]},
                        }
                    )
                si = dict(si)
                si["on_wait"] = keep
                inst = dict(inst)
                inst["sync_info"] = si
            out.append(inst)
        blk["instructions"] = out

    def walk(o):
        if isinstance(o, dict):
            if "instructions" in o:
                fix_block(o)
            for v in o.values():
                walk(v)
        elif isinstance(o, list):
            for v in o:
                walk(v)

    walk(m)
    return json.dumps(m).encode()


_orig_compile_bir_kernel = bass_utils.compile_bir_kernel


def _legalizing_compile_bir_kernel(bir_json, *args, **kwargs):
    if isinstance(bir_json, str):
        bir_json = bir_json.encode()
    return _orig_compile_bir_kernel(_legalize_bir(bir_json), *args, **kwargs)


if bass_utils.compile_bir_kernel is not _legalizing_compile_bir_kernel:
    bass_utils.compile_bir_kernel = _legalizing_compile_bir_kernel
    bass2jax.compile_bir_kernel = _legalizing_compile_bir_kernel

# positions = int32(jnp.linspace(0, L-2K-1, H)) for L=131072, K=1024, H=128
# (verified identical to the jax reference's values)
POS = [
    0, 1015, 2031, 3047, 4063, 5079, 6095, 7111, 8127, 9143, 10159, 11175,
    12191, 13207, 14223, 15238, 16254, 17270, 18286, 19302, 20318, 21334,
    22350, 23366, 24382, 25398, 26414, 27430, 28446, 29461, 30477, 31493,
    32509, 33525, 34541, 35557, 36573, 37589, 38605, 39621, 40637, 41653,
    42669, 43684, 44700, 45716, 46732, 47748, 48764, 49780, 50796, 51812,
    52828, 53844, 54860, 55876, 56892, 57907, 58923, 59939, 60955, 61971,
    62987, 64003, 65019, 66035, 67051, 68067, 69083, 70099, 71115, 72130,
    73146, 74162, 75178, 76194, 77210, 78226, 79242, 80258, 81274, 82290,
    83306, 84322, 85338, 86353, 87369, 88385, 89401, 90417, 91433, 92449,
    93465, 94481, 95497, 96513, 97529, 98545, 99561, 100576, 101592, 102608,
    103624, 104640, 105656, 106672, 107688, 108704, 109720, 110736, 111752,
    112768, 113784, 114799, 115815, 116831, 117847, 118863, 119879, 120895,
    121911, 122927, 123943, 124959, 125975, 126991, 128007, 129023,
]

N_CORES = 8
B_FULL, C_DIM, L_DIM = 16, 8, 131072
K_DIM, H_DIM = 1024, 128
B_LOC = B_FULL // N_CORES          # batches per core
N_GROUPS = 2 * B_LOC               # 4 groups of 512 rows; row = b*1024+h*8+c
F32 = mybir.dt.float32
F32R = mybir.dt.float32r
BF16 = mybir.dt.bfloat16
FP8 = mybir.dt.float8e4
NP_FP8 = ml_dtypes.float8_e4m3fn
NP_BF16 = ml_dtypes.bfloat16
DR = mybir.MatmulPerfMode.DoubleRow

_prog_cache = {}


def make_constants():
    K = K_DIM
    hann = 0.5 * (1.0 - np.cos(2.0 * np.pi * np.arange(K) / K))
    # packed spectrum bins: j = 0..512 -> cos bin j; j = 513..1023 -> sin
    # bin (j - 512). 513 cos + 511 sin = exactly 1024 columns.
    j = np.arange(K)
    m_of_j = np.where(j <= 512, j, j - 512)
    is_sin = j > 512
    k = np.arange(K)[:, None].astype(np.float64)
    ang = 2 * np.pi * k * m_of_j[None, :] / K
    cs1 = hann[:, None] * np.where(is_sin[None, :], np.sin(ang), np.cos(ang))
    # DMA-friendly layout: [bin-chunk q][p][ktile t][j-in-chunk] so each
    # per-bin-chunk DMA reads one contiguous 128KB block.
    cs1 = cs1.reshape(8, 128, 8, 128)            # [t][p][q][jin]
    cs1 = np.ascontiguousarray(cs1.transpose(2, 1, 0, 3))  # [q][p][t][jin]
    cs1 = cs1.astype(np.float32).astype(NP_FP8)

    # irfft weights on packed bins, x64 to undo the (w/8)^2 scaling.
    # acov[d] = sum_j wt[m_j] * P[j] * cos(2*pi*m_j*d/K) / K,  d = 0..512
    wt = np.where((m_of_j == 0) | (m_of_j == 512), 1.0, 2.0)
    d = np.arange(513)[None, :]
    c2 = (wt[:, None] * np.cos(2 * np.pi * m_of_j[:, None] * d / K) / K) * 64.0
    c2 = np.concatenate([c2, np.zeros((K, 7))], axis=1)  # pad 513 -> 520
    c2 = c2.reshape(8, 128, 520)                 # [t][p][d] ; bin = 128t + p
    c2 = np.ascontiguousarray(c2.transpose(1, 0, 2))      # [p][t][d]
    c2 = c2.astype(np.float32).astype(NP_FP8)

    ident = np.eye(128, dtype=np.float32)

    # gather start offsets (elements into flat [B_LOC*C*L] x) per tile row:
    # tile j = g*4 + i; row p = hh*8 + c; h = (g%2)*64 + 16*i + hh
    gidx = np.zeros((16, 128), dtype=np.int32)
    for g in range(N_GROUPS):
        b = g // 2
        h0 = (g % 2) * 64
        for i in range(4):
            for hh in range(16):
                h = h0 + 16 * i + hh
                for c in range(C_DIM):
                    gidx[g * 4 + i, hh * 8 + c] = (
                        b * C_DIM * L_DIM + c * L_DIM + POS[h]
                    )
    gidx_t = np.ascontiguousarray(gidx.T)        # [p][tile]
    return {"cs1": cs1, "c2": c2, "ident": ident, "gidx": gidx_t}


def build_program():
    nc = bass.Bass("TRN2", target_bir_lowering=False, debug=False,
                   num_swdge_queues=4)
    x = nc.dram_tensor("x", [B_LOC, C_DIM, L_DIM], F32R,
                       kind="ExternalInput").ap()
    cs1 = nc.dram_tensor("cs1", [8, 128, 8, 128], FP8,
                         kind="ExternalInput").ap()
    c2 = nc.dram_tensor("c2", [128, 8, 520], FP8, kind="ExternalInput").ap()
    ident = nc.dram_tensor("ident", [128, 128], F32R,
                           kind="ExternalInput").ap()
    gidx = nc.dram_tensor("gidx", [128, 16], mybir.dt.int32,
                          kind="ExternalInput").ap()
    y = nc.dram_tensor(
        "y", [B_LOC, H_DIM, C_DIM, K_DIM], BF16, kind="ExternalOutput"
    ).ap()

    with tile.TileContext(nc) as tc:
        with (
            tc.tile_pool(name="singles", bufs=1) as singles,
            tc.tile_pool(name="gather", bufs=16) as gpool,
            tc.tile_pool(name="wt", bufs=2) as wtpool,
            tc.tile_pool(name="pp", bufs=2) as ppool,
            tc.tile_pool(name="yy", bufs=4) as ypool,
            tc.tile_pool(name="small", bufs=48) as smallpool,
            tc.tile_pool(name="tp_ps", bufs=2, space="PSUM") as tp_ps_pool,
            tc.tile_pool(name="mm1_ps", bufs=4, space="PSUM") as mm1_ps_pool,
            tc.tile_pool(name="mm2_ps", bufs=1, space="PSUM") as mm2_ps_pool,
        ):
            # small control inputs first so gathers start immediately
            gidx_sb = singles.tile([128, 16], mybir.dt.int32)
            nc.sync.dma_start(out=gidx_sb, in_=gidx)
            id_sb = singles.tile([128, 128], F32R)
            nc.sync.dma_start(out=id_sb, in_=ident)
            x_flat = x.rearrange("b c l -> (b c) l")

            # all 16 gathers up front, spread over the 8 SWDGE queues
            gts = []
            inv2s = []
            for jt in range(16):
                gt = gpool.tile([128, K_DIM], F32R, tag="gt")
                gd = nc.gpsimd.indirect_dma_start(
                    out=gt[:],
                    out_offset=None,
                    in_=x_flat,
                    in_offset=bass.IndirectOffsetOnAxis(
                        ap=gidx_sb[:, jt : jt + 1], axis=1
                    ),
                )
                qi = jt % 4
                if qi:
                    gd.ins.queue = f"qPoolDynamic{qi}"
                fm = smallpool.tile([128, 1], F32, tag="fm")
                nc.vector.reduce_max(
                    out=fm, in_=gt,
                    axis=mybir.AxisListType.X,
                    apply_absolute_value=True,
                )
                inv = smallpool.tile([128, 1], F32, tag="inv")
                nc.vector.reciprocal(out=inv, in_=fm)
                inv2 = smallpool.tile([128, 1], F32, tag="inv2")
                nc.vector.tensor_mul(inv2, inv, inv)
                gts.append(gt)
                inv2s.append(inv2)

            # DFT matrices (fp8): cs1 per bin-chunk (contiguous 128KB each),
            # c2 in one DMA
            cs1_sb = singles.tile([128, 8, K_DIM], FP8)
            for q in range(8):
                nc.sync.dma_start(
                    out=cs1_sb[:, :, 128 * q : 128 * (q + 1)], in_=cs1[q]
                )
            c2_sb = singles.tile([128, 8, 520], FP8)
            nc.sync.dma_start(out=c2_sb, in_=c2)

            def mm1_sub(g, wt_t, p_sb, q, half):
                # one spectrum sub-tile [bin(64), row(512)], fp8 DoubleRow
                # (DR outputs must start at PSUM partition 0); the square
                # writes it into the 128-partition p_sb with a partition
                # shift, casting to fp8. ~14/16 of the squares go to ACT,
                # the rest to DVE (load balance).
                mq = mm1_ps_pool.tile([64, 512], F32, tag="mm1")
                b0 = 128 * q + 64 * half
                for c in range(4):
                    nc.tensor.matmul(
                        mq,
                        cs1_sb[:, 2 * c : 2 * c + 2, b0 : b0 + 64],
                        wt_t[:, 2 * c : 2 * c + 2, :],
                        start=(c == 0),
                        stop=(c == 3),
                        perf_mode=DR,
                    )
                dst = p_sb[64 * half : 64 * (half + 1), q, :]
                nc.scalar.square(dst, mq)

            def mm2_rt(g, p_sb, rp):
                # acov[row, 0:513] = P.T @ c2 for rows 128*rp..+127, plain
                # fp8 matmul (full 128-partition output); then expand by
                # symmetry: y[512+d] = acov[d], y[j] = acov[512-j]; x 1/fmax^2
                b = g // 2
                h0 = (g % 2) * 64
                yp = mm2_ps_pool.tile([128, 520], F32, tag="mm2")
                r0 = 128 * rp
                for t in range(8):
                    nc.tensor.matmul(
                        yp[:, 0:512],
                        p_sb[:, t, r0 : r0 + 128],
                        c2_sb[:, t, 0:512],
                        start=(t == 0),
                        stop=(t == 7),
                    )
                    # moving free dim caps at 512 (s3d3_mm_num_elements), so
                    # the d=512 column rides in a second 8-col group sharing
                    # this chunk's stationary tile
                    nc.tensor.matmul(
                        yp[:, 512:520],
                        p_sb[:, t, r0 : r0 + 128],
                        c2_sb[:, t, 512:520],
                        start=(t == 0),
                        stop=(t == 7),
                    )
                ysb = ypool.tile([128, K_DIM], BF16, tag="y")
                inv2 = inv2s[4 * g + rp]
                nc.vector.tensor_scalar_mul(
                    ysb[:, 512:1024], yp[:, 0:512], inv2
                )
                rev = bass.AP(
                    tensor=yp.tensor,
                    offset=yp.offset + 512,
                    ap=[list(yp.ap[0]), [-1, 512]],
                )
                nc.vector.tensor_scalar_mul(ysb[:, 0:512], rev, inv2)
                hs = h0 + 16 * rp
                dst = y[b, hs : hs + 16].rearrange("h c n -> (h c) n")
                nc.sync.dma_start(out=dst, in_=ysb)

            # software pipeline: mm2(g-1) rowtiles are interleaved between
            # transpose chunks and mm1 subs of group g, so the single
            # mm2 PSUM buffer never stalls the PE (y-copies drain on DVE
            # while the PE runs the next chunk).
            prev = None            # (g-1, its p_sb)
            for g in range(N_GROUPS):
                p_sb = ppool.tile([128, 8, 512], FP8, tag="p")
                wt_t = wtpool.tile([128, 8, 512], FP8, tag="wt")
                for t in range(8):
                    tp = tp_ps_pool.tile([128, 512], F32R, tag="tp")
                    for i in range(4):
                        nc.tensor.transpose(
                            tp[:, 128 * i : 128 * (i + 1)],
                            gts[4 * g + i][:, 128 * t : 128 * (t + 1)],
                            id_sb,
                        )
                    nc.scalar.mul(wt_t[:, t, :], tp, 0.125)
                    if t == 3 and prev is not None:
                        mm2_rt(*prev, 0)
                    if t == 7 and prev is not None:
                        mm2_rt(*prev, 1)
                for q in range(8):
                    for half in range(2):
                        mm1_sub(g, wt_t, p_sb, q, half)
                    if q == 3 and prev is not None:
                        mm2_rt(*prev, 2)
                    if q == 7 and prev is not None:
                        mm2_rt(*prev, 3)
                prev = (g, p_sb)
            for rp in range(4):
                mm2_rt(*prev, rp)
    return nc


def get_program():
    if "nc" not in _prog_cache:
        _prog_cache["nc"] = build_program()
        _prog_cache["consts"] = make_constants()
    return _prog_cache["nc"], _prog_cache["consts"]


def kernel(X, kernel_size=None, out_channels=None, _trace=False):
    X = np.ascontiguousarray(np.asarray(X, dtype=np.float32))
    assert X.shape == (B_FULL, C_DIM, L_DIM)
    nc, consts = get_program()
    in_maps = []
    for c in range(N_CORES):
        m = {"x": X[c * B_LOC : (c + 1) * B_LOC]}
        m.update(consts)
        in_maps.append(m)
    res = run_bass_kernel_spmd(
        nc, in_maps, core_ids=list(range(N_CORES)), trace=_trace
    )
    out = np.concatenate(
        [np.asarray(r["y"]).astype(np.float32) for r in res.results], axis=0
    )
    if _trace:
        return out, res
    return out


# revision 12
# speedup vs baseline: 1.3029x; 1.1743x over previous
"""Trainium2 Bass kernel for LocalPatternFilter.

Reference computation (per (b, h, c) row of length K=1024):
  1. gather window  g = X[b, c, pos[h] : pos[h]+K]
  2. fmax = max|g|;  w = g * hann / fmax
  3. acov = ifftshift(irfft(|rfft(w)|^2))   (= circular autocorrelation)

Implemented as dense DFT matmuls on the tensor engine in bf16 (PE streams
1 moving column/cycle for every dtype <= 2B, so bf16 matches fp8 speed at
much better accuracy):
  - packed 1024-bin spectrum: cols [c0..c512, s1..s511] (513 cos + 511 sin
    = exactly 1024 bins, hann folded into the matrix)
  - squares -> bf16 scratch; pair-add P[m] = c_m^2 + s_m^2 on DVE shrinks
    the irfft contraction to 512 bins (+ a 1-row correction for the c512^2
    term that pollutes P[0], and a partition-folded Q vector that yields
    the 513th output column acov[512] with a single tiny matmul)
  - irfft + ifftshift + output symmetry folded into c2; the mirrored half
    of each output row is written via a reversed-stride PSUM read.
  - 1/fmax^2 folded into the PSUM->SBUF copy (acov is linear in P); the
    gather DMA casts to bf16 (software DGE cast), fmax computed from the
    bf16 data.
  - output stored bf16, converted to f32 on host.

Sharding: data-parallel over batch, 2 batches per core on 8 cores.
"""

import json

import numpy as np
import ml_dtypes

import concourse.bass as bass
import concourse.bass2jax as bass2jax
import concourse.bass_utils as bass_utils
import concourse.tile as tile
from concourse import mybir
from concourse.bass_utils import run_bass_kernel_spmd

# ---------------------------------------------------------------------------
# The walrus build in this container accepts at most ONE sync-wait command per
# TPB instruction ("Too many sync wait commands" in setupSyncWait), while Tile
# emits several (multi-queue DMA deps, the kernel-tail drain). Legalize the
# serialized BIR before compiling: hoist excess waits onto preceding
# same-engine wait-only EventSemaphore instructions. Engines execute their
# instruction streams in order, so this is semantically identical.
# ---------------------------------------------------------------------------
_MAX_WAITS = 1


def _legalize_bir(bir_bytes):
    m = json.loads(bir_bytes)
    counter = [0]

    def fix_block(blk):
        insts = blk.get("instructions")
        if not isinstance(insts, list):
            return
        out = []
        for inst in insts:
            si = inst.get("sync_info") or {}
            waits = si.get("on_wait") or []
            if isinstance(inst.get("opcode"), str) and len(waits) > _MAX_WAITS:
                keep = waits[-_MAX_WAITS:]
                for w in waits[:-_MAX_WAITS]:
                    counter[0] += 1
                    out.append(
                        {
                            "debug": inst.get("debug", 0),
                            "engine": inst["engine"],
                            "ins": [],
                            "name": f"LGW-{counter[0]}-{inst['name']}",
                            "opcode": "EventSemaphore",
                            "outs": [],
                            "sync_info": {"on_update": [], "on_wait": [w]},
                        }
                    )
                si = dict(si)
                si["on_wait"] = keep
                inst = dict(inst)
                inst["sync_info"] = si
            out.append(inst)
        blk["instructions"] = out

    def walk(o):
        if isinstance(o, dict):
            if "instructions" in o:
                fix_block(o)
            for v in o.values():
                walk(v)
        elif isinstance(o, list):
            for v in o:
                walk(v)

    walk(m)
    return json.dumps(m).encode()


_orig_compile_bir_kernel = bass_utils.compile_bir_kernel


def _legalizing_compile_bir_kernel(bir_json, *args, **kwargs):
    if isinstance(bir_json, str):
        bir_json = bir_json.encode()
    return _orig_compile_bir_kernel(_legalize_bir(bir_json), *args, **kwargs)


if bass_utils.compile_bir_kernel is not _legalizing_compile_bir_kernel:
    bass_utils.compile_bir_kernel = _legalizing_compile_bir_kernel
    bass2jax.compile_bir_kernel = _legalizing_compile_bir_kernel

# positions = int32(jnp.linspace(0, L-2K-1, H)) for L=131072, K=1024, H=128
# (verified identical to the jax reference's values)
POS = [
    0, 1015, 2031, 3047, 4063, 5079, 6095, 7111, 8127, 9143, 10159, 11175,
    12191, 13207, 14223, 15238, 16254, 17270, 18286, 19302, 20318, 21334,
    22350, 23366, 24382, 25398, 26414, 27430, 28446, 29461, 30477, 31493,
    32509, 33525, 34541, 35557, 36573, 37589, 38605, 39621, 40637, 41653,
    42669, 43684, 44700, 45716, 46732, 47748, 48764, 49780, 50796, 51812,
    52828, 53844, 54860, 55876, 56892, 57907, 58923, 59939, 60955, 61971,
    62987, 64003, 65019, 66035, 67051, 68067, 69083, 70099, 71115, 72130,
    73146, 74162, 75178, 76194, 77210, 78226, 79242, 80258, 81274, 82290,
    83306, 84322, 85338, 86353, 87369, 88385, 89401, 90417, 91433, 92449,
    93465, 94481, 95497, 96513, 97529, 98545, 99561, 100576, 101592, 102608,
    103624, 104640, 105656, 106672, 107688, 108704, 109720, 110736, 111752,
    112768, 113784, 114799, 115815, 116831, 117847, 118863, 119879, 120895,
    121911, 122927, 123943, 124959, 125975, 126991, 128007, 129023,
]

N_CORES = 8
B_FULL, C_DIM, L_DIM = 16, 8, 131072
K_DIM, H_DIM = 1024, 128
B_LOC = B_FULL // N_CORES          # batches per core
N_GROUPS = 2 * B_LOC               # 4 groups of 512 rows; row = b*1024+h*8+c
F32 = mybir.dt.float32
BF16 = mybir.dt.bfloat16
NP_BF16 = ml_dtypes.bfloat16

_prog_cache = {}


def make_constants():
    K = K_DIM
    hann = 0.5 * (1.0 - np.cos(2.0 * np.pi * np.arange(K) / K))
    # packed spectrum bins: j = 0..512 -> cos bin j; j = 513..1023 -> sin
    # bin (j - 512). 513 cos + 511 sin = exactly 1024 columns.
    j = np.arange(K)
    m_of_j = np.where(j <= 512, j, j - 512)
    is_sin = j > 512
    k = np.arange(K)[:, None].astype(np.float64)
    ang = 2 * np.pi * k * m_of_j[None, :] / K
    cs1 = hann[:, None] * np.where(is_sin[None, :], np.sin(ang), np.cos(ang))
    # DMA-friendly layout: [bin-chunk q][p][ktile t][j-in-chunk] so each
    # per-bin-chunk DMA reads one contiguous 256KB block.
    cs1 = cs1.reshape(8, 128, 8, 128)            # [t][p][q][jin]
    cs1 = np.ascontiguousarray(cs1.transpose(2, 1, 0, 3))  # [q][p][t][jin]
    cs1 = cs1.astype(np.float32).astype(NP_BF16)

    # irfft on pair-added bins: acov[d] = sum_{m<512} wt[m] P[m] cos(..)/K
    # + correction (chunk 4) for c512^2 riding in P[0]: (cos(pi d) - 1)/K
    # at partition 0 of the q=4 squares tile.
    wt = np.where(np.arange(512) == 0, 1.0, 2.0)
    d = np.arange(513)[None, :]
    m = np.arange(512)[:, None]
    c2a = wt[:, None] * np.cos(2 * np.pi * m * d / K) / K      # (512, 513)
    c2cor = np.zeros((128, 513))
    c2cor[0, :] = (np.cos(np.pi * d[0]) - 1.0) / K
    c2 = np.concatenate([c2a.reshape(4, 128, 513),
                         c2cor[None, :, :]], axis=0)           # (5, 128, 513)
    c2 = np.concatenate([c2, np.zeros((5, 128, 7))], axis=2)   # pad -> 520
    c2 = np.ascontiguousarray(c2.transpose(1, 0, 2))           # [p][t][d]
    c2 = c2.astype(np.float32).astype(NP_BF16)

    # acov[512] = (2/K) sum_p (-1)^p Q[p], with Q[p] = sum_t a_t[p] P[128t+p]
    # and a weight of 1/2 on P_add[0] only (its wt is 1, not 2; it already
    # contains c512^2, whose cos(pi*512) = 1 coefficient matches P[0]'s).
    # (-1)^m = (-1)^p since m = 128t + p. Output column 0 of an 8-wide tiny
    # matmul; columns 1..7 are zero pad.
    c2q = np.zeros((128, 8))
    p = np.arange(128)
    c2q[:, 0] = 2.0 * ((-1.0) ** p) / K
    c2q = c2q.astype(np.float32).astype(NP_BF16)
    w0 = np.ones((128, 1), dtype=np.float32)
    w0[0, 0] = 0.5

    ident = np.eye(128, dtype=np.float32).astype(NP_BF16)

    # gather start offsets (elements into flat [B_LOC*C*L] x) per tile row:
    # tile j = g*4 + i; row p = hh*8 + c; h = (g%2)*64 + 16*i + hh
    gidx = np.zeros((16, 128), dtype=np.int32)
    for g in range(N_GROUPS):
        b = g // 2
        h0 = (g % 2) * 64
        for i in range(4):
            for hh in range(16):
                h = h0 + 16 * i + hh
                for c in range(C_DIM):
                    gidx[g * 4 + i, hh * 8 + c] = (
                        b * C_DIM * L_DIM + c * L_DIM + POS[h]
                    )
    gidx_t = np.ascontiguousarray(gidx.T)        # [p][tile]
    return {"cs1": cs1, "c2": c2, "c2q": c2q, "w0": w0, "ident": ident,
            "gidx": gidx_t}


def build_program():
    nc = bass.Bass("TRN2", target_bir_lowering=False, debug=False,
                   num_swdge_queues=4)
    x = nc.dram_tensor("x", [B_LOC, C_DIM, L_DIM], F32,
                       kind="ExternalInput").ap()
    cs1 = nc.dram_tensor("cs1", [8, 128, 8, 128], BF16,
                         kind="ExternalInput").ap()
    c2 = nc.dram_tensor("c2", [128, 5, 520], BF16, kind="ExternalInput").ap()
    c2q = nc.dram_tensor("c2q", [128, 8], BF16, kind="ExternalInput").ap()
    w0 = nc.dram_tensor("w0", [128, 1], F32, kind="ExternalInput").ap()
    ident = nc.dram_tensor("ident", [128, 128], BF16,
                           kind="ExternalInput").ap()
    gidx = nc.dram_tensor("gidx", [128, 16], mybir.dt.int32,
                          kind="ExternalInput").ap()
    y = nc.dram_tensor(
        "y", [B_LOC, H_DIM, C_DIM, K_DIM], BF16, kind="ExternalOutput"
    ).ap()

    with tile.TileContext(nc) as tc:
        with (
            tc.tile_pool(name="singles", bufs=1) as singles,
            tc.tile_pool(name="gather", bufs=16) as gpool,
            tc.tile_pool(name="wt", bufs=2) as wtpool,
            tc.tile_pool(name="sq", bufs=2) as sqpool,
            tc.tile_pool(name="pa", bufs=2) as papool,
            tc.tile_pool(name="yy", bufs=4) as ypool,
            tc.tile_pool(name="small", bufs=48) as smallpool,
            tc.tile_pool(name="tp_ps", bufs=2, space="PSUM") as tp_ps_pool,
            tc.tile_pool(name="mm1_ps", bufs=2, space="PSUM") as mm1_ps_pool,
            tc.tile_pool(name="mm2_ps", bufs=2, space="PSUM") as mm2_ps_pool,
        ):
            # small control inputs first so gathers start immediately
            gidx_sb = singles.tile([128, 16], mybir.dt.int32)
            nc.sync.dma_start(out=gidx_sb, in_=gidx)
            id_sb = singles.tile([128, 128], BF16)
            nc.sync.dma_start(out=id_sb, in_=ident)
            x_flat = x.rearrange("b c l -> (b c) l")

            # all 16 gathers up front, spread over the 4 SWDGE queues;
            # software DGE casts f32 -> bf16 on the fly
            gts = []
            inv2s = []
            for jt in range(16):
                gt = gpool.tile([128, K_DIM], BF16, tag="gt")
                gd = nc.gpsimd.indirect_dma_start(
                    out=gt[:],
                    out_offset=None,
                    in_=x_flat,
                    in_offset=bass.IndirectOffsetOnAxis(
                        ap=gidx_sb[:, jt : jt + 1], axis=1
                    ),
                )
                qi = jt % 4
                if qi:
                    gd.ins.queue = f"qPoolDynamic{qi}"
                fm = smallpool.tile([128, 1], F32, tag="fm")
                nc.vector.reduce_max(
                    out=fm, in_=gt,
                    axis=mybir.AxisListType.X,
                    apply_absolute_value=True,
                )
                inv = smallpool.tile([128, 1], F32, tag="inv")
                nc.vector.reciprocal(out=inv, in_=fm)
                inv2 = smallpool.tile([128, 1], F32, tag="inv2")
                nc.vector.tensor_mul(inv2, inv, inv)
                gts.append(gt)
                inv2s.append(inv2)

            # DFT matrices (bf16): cs1 per bin-chunk (contiguous 256KB each)
            cs1_sb = singles.tile([128, 8, K_DIM], BF16)
            for q in range(8):
                nc.sync.dma_start(
                    out=cs1_sb[:, :, 128 * q : 128 * (q + 1)], in_=cs1[q]
                )
            c2_sb = singles.tile([128, 5, 520], BF16)
            nc.sync.dma_start(out=c2_sb, in_=c2)
            c2q_sb = singles.tile([128, 8], BF16)
            nc.sync.dma_start(out=c2q_sb, in_=c2q)
            w0_sb = singles.tile([128, 1], F32)
            nc.sync.dma_start(out=w0_sb, in_=w0)

            def mm1_q(g, wt_t, sq, q):
                # one spectrum tile [bin(128), row(512)] + square to scratch
                mq = mm1_ps_pool.tile([128, 512], F32, tag="mm1")
                for t in range(8):
                    nc.tensor.matmul(
                        mq,
                        cs1_sb[:, t, 128 * q : 128 * (q + 1)],
                        wt_t[:, t, :],
                        start=(t == 0),
                        stop=(t == 7),
                    )
                nc.scalar.square(sq[:, q, :], mq)

            def mm2_rt(g, pa, sq, qv, rp):
                # acov[row, 0:513] for rows 128*rp..+127: 4 chunks over the
                # pair-added P + 1 correction chunk (c512^2 rides in P[0]) +
                # a tiny matmul over Q for the acov[512] column; then expand
                # by symmetry: y[512+d] = acov[d], y[j] = acov[512-j], and
                # scale by 1/fmax^2.
                b = g // 2
                h0 = (g % 2) * 64
                yp = mm2_ps_pool.tile([128, 520], F32, tag="mm2")
                r0 = 128 * rp
                for t in range(4):
                    nc.tensor.matmul(
                        yp[:, 0:512],
                        pa[:, t, r0 : r0 + 128],
                        c2_sb[:, t, 0:512],
                        start=(t == 0),
                        stop=False,
                    )
                nc.tensor.matmul(
                    yp[:, 0:512],
                    sq[:, 4, r0 : r0 + 128],
                    c2_sb[:, 4, 0:512],
                    start=False,
                    stop=True,
                )
                # d = 512 column (plus 7 pad): single 128-deep contraction
                # over the partition-folded Q
                nc.tensor.matmul(
                    yp[:, 512:520],
                    qv[:, r0 : r0 + 128],
                    c2q_sb,
                    start=True,
                    stop=True,
                )
                ysb = ypool.tile([128, K_DIM], BF16, tag="y")
                inv2 = inv2s[4 * g + rp]
                nc.vector.tensor_scalar_mul(
                    ysb[:, 512:1024], yp[:, 0:512], inv2
                )
                rev = bass.AP(
                    tensor=yp.tensor,
                    offset=yp.offset + 512,
                    ap=[list(yp.ap[0]), [-1, 512]],
                )
                nc.vector.tensor_scalar_mul(ysb[:, 0:512], rev, inv2)
                hs = h0 + 16 * rp
                dst = y[b, hs : hs + 16].rearrange("h c n -> (h c) n")
                nc.sync.dma_start(out=dst, in_=ysb)

            # software pipeline: mm2(g-1) rowtiles are interleaved between
            # transpose chunks and mm1 tiles of group g so y-copies drain
            # on DVE while the PE keeps streaming.
            prev = None            # (g-1, pa, sq, qv)
            for g in range(N_GROUPS):
                wt_t = wtpool.tile([128, 8, 512], BF16, tag="wt")
                sq = sqpool.tile([128, 8, 512], BF16, tag="sq")
                pa = papool.tile([128, 4, 512], BF16, tag="pa")
                qv = papool.tile([128, 512], BF16, tag="qv")
                for t in range(8):
                    tp = tp_ps_pool.tile([128, 512], BF16, tag="tp")
                    for i in range(4):
                        nc.tensor.transpose(
                            tp[:, 128 * i : 128 * (i + 1)],
                            gts[4 * g + i][:, 128 * t : 128 * (t + 1)],
                            id_sb,
                        )
                    nc.scalar.copy(out=wt_t[:, t, :], in_=tp)
                    if t == 3 and prev is not None:
                        mm2_rt(*prev, 0)
                    if t == 7 and prev is not None:
                        mm2_rt(*prev, 1)
                for q in range(8):
                    mm1_q(g, wt_t, sq, q)
                    if q == 3 and prev is not None:
                        mm2_rt(*prev, 2)
                    if q == 7 and prev is not None:
                        mm2_rt(*prev, 3)
                # pair-add P[m] = c_m^2 + s_m^2 and partition-folded Q
                for t in range(4):
                    nc.vector.tensor_add(pa[:, t, :], sq[:, t, :],
                                         sq[:, 4 + t, :])
                nc.vector.scalar_tensor_tensor(
                    qv, pa[:, 0, :], w0_sb, pa[:, 1, :],
                    op0=mybir.AluOpType.mult, op1=mybir.AluOpType.add,
                )
                nc.vector.tensor_add(qv, qv, pa[:, 2, :])
                nc.vector.tensor_add(qv, qv, pa[:, 3, :])
                prev = (g, pa, sq, qv)
            for rp in range(4):
                mm2_rt(*prev, rp)
    return nc


def get_program():
    if "nc" not in _prog_cache:
        _prog_cache["nc"] = build_program()
        _prog_cache["consts"] = make_constants()
    return _prog_cache["nc"], _prog_cache["consts"]


def kernel(X, kernel_size=None, out_channels=None, _trace=False):
    X = np.ascontiguousarray(np.asarray(X, dtype=np.float32))
    assert X.shape == (B_FULL, C_DIM, L_DIM)
    nc, consts = get_program()
    in_maps = []
    for c in range(N_CORES):
        m = {"x": X[c * B_LOC : (c + 1) * B_LOC]}
        m.update(consts)
        in_maps.append(m)
    res = run_bass_kernel_spmd(
        nc, in_maps, core_ids=list(range(N_CORES)), trace=_trace
    )
    out = np.concatenate(
        [np.asarray(r["y"]).astype(np.float32) for r in res.results], axis=0
    )
    if _trace:
        return out, res
    return out


# revision 27
# speedup vs baseline: 1.4039x; 1.0775x over previous
"""Trainium2 Bass kernel for LocalPatternFilter.

Reference computation (per (b, h, c) row of length K=1024):
  1. gather window  g = X[b, c, pos[h] : pos[h]+K]
  2. fmax = max|g|;  w = g * hann / fmax
  3. acov = ifftshift(irfft(|rfft(w)|^2))   (= circular autocorrelation)

Implemented as dense DFT matmuls on the tensor engine in bf16 (PE streams
1 moving column/cycle for every dtype <= 2B, so bf16 matches fp8 speed at
much better accuracy):
  - packed 1024-bin spectrum: cols [c0..c512, s1..s511] (513 cos + 511 sin
    = exactly 1024 bins, hann folded into the matrix)
  - squares -> bf16 scratch; pair-add P[m] = c_m^2 + s_m^2 on DVE shrinks
    the irfft contraction to 512 bins (+ a 1-row correction for the c512^2
    term that pollutes P[0], and a partition-folded Q vector that yields
    the 513th output column acov[512] with a single tiny matmul)
  - irfft + ifftshift + output symmetry folded into c2; the mirrored half
    of each output row is written via a reversed-stride PSUM read.
  - 1/fmax^2 folded into the PSUM->SBUF copy (acov is linear in P); the
    gather DMA casts to bf16 (software DGE cast), fmax computed from the
    bf16 data.
  - output stored bf16, converted to f32 on host.

Sharding: data-parallel over batch, 2 batches per core on 8 cores.
"""

import json

import numpy as np
import ml_dtypes

import concourse.bass as bass
import concourse.bass2jax as bass2jax
import concourse.bass_utils as bass_utils
import concourse.tile as tile
from concourse import mybir
from concourse.bass_utils import run_bass_kernel_spmd

# ---------------------------------------------------------------------------
# The walrus build in this container accepts at most ONE sync-wait command per
# TPB instruction ("Too many sync wait commands" in setupSyncWait), while Tile
# emits several (multi-queue DMA deps, the kernel-tail drain). Legalize the
# serialized BIR before compiling: hoist excess waits onto preceding
# same-engine wait-only EventSemaphore instructions. Engines execute their
# instruction streams in order, so this is semantically identical.
# ---------------------------------------------------------------------------
_MAX_WAITS = 1


def _legalize_bir(bir_bytes):
    m = json.loads(bir_bytes)
    counter = [0]

    def fix_block(blk):
        insts = blk.get("instructions")
        if not isinstance(insts, list):
            return
        out = []
        for inst in insts:
            si = inst.get("sync_info") or {}
            waits = si.get("on_wait") or []
            if isinstance(inst.get("opcode"), str) and len(waits) > _MAX_WAITS:
                keep = waits[-_MAX_WAITS:]
                for w in waits[:-_MAX_WAITS]:
                    counter[0] += 1
                    out.append(
                        {
                            "debug": inst.get("debug", 0),
                            "engine": inst["engine"],
                            "ins": [],
                            "name": f"LGW-{counter[0]}-{inst['name']}",
                            "opcode": "EventSemaphore",
                            "outs": [],
                            "sync_info": {"on_update": [], "on_wait": [w]},
                        }
                    )
                si = dict(si)
                si["on_wait"] = keep
                inst = dict(inst)
                inst["sync_info"] = si
            out.append(inst)
        blk["instructions"] = out

    def walk(o):
        if isinstance(o, dict):
            if "instructions" in o:
                fix_block(o)
            for v in o.values():
                walk(v)
        elif isinstance(o, list):
            for v in o:
                walk(v)

    walk(m)
    return json.dumps(m).encode()


_orig_compile_bir_kernel = bass_utils.compile_bir_kernel


def _legalizing_compile_bir_kernel(bir_json, *args, **kwargs):
    if isinstance(bir_json, str):
        bir_json = bir_json.encode()
    return _orig_compile_bir_kernel(_legalize_bir(bir_json), *args, **kwargs)


if bass_utils.compile_bir_kernel is not _legalizing_compile_bir_kernel:
    bass_utils.compile_bir_kernel = _legalizing_compile_bir_kernel
    bass2jax.compile_bir_kernel = _legalizing_compile_bir_kernel

# positions = int32(jnp.linspace(0, L-2K-1, H)) for L=131072, K=1024, H=128
# (verified identical to the jax reference's values)
POS = [
    0, 1015, 2031, 3047, 4063, 5079, 6095, 7111, 8127, 9143, 10159, 11175,
    12191, 13207, 14223, 15238, 16254, 17270, 18286, 19302, 20318, 21334,
    22350, 23366, 24382, 25398, 26414, 27430, 28446, 29461, 30477, 31493,
    32509, 33525, 34541, 35557, 36573, 37589, 38605, 39621, 40637, 41653,
    42669, 43684, 44700, 45716, 46732, 47748, 48764, 49780, 50796, 51812,
    52828, 53844, 54860, 55876, 56892, 57907, 58923, 59939, 60955, 61971,
    62987, 64003, 65019, 66035, 67051, 68067, 69083, 70099, 71115, 72130,
    73146, 74162, 75178, 76194, 77210, 78226, 79242, 80258, 81274, 82290,
    83306, 84322, 85338, 86353, 87369, 88385, 89401, 90417, 91433, 92449,
    93465, 94481, 95497, 96513, 97529, 98545, 99561, 100576, 101592, 102608,
    103624, 104640, 105656, 106672, 107688, 108704, 109720, 110736, 111752,
    112768, 113784, 114799, 115815, 116831, 117847, 118863, 119879, 120895,
    121911, 122927, 123943, 124959, 125975, 126991, 128007, 129023,
]

N_CORES = 8
B_FULL, C_DIM, L_DIM = 16, 8, 131072
K_DIM, H_DIM = 1024, 128
B_LOC = B_FULL // N_CORES          # batches per core
N_GROUPS = 8                       # 8 groups of 256 rows (2 gather tiles)
F32 = mybir.dt.float32
BF16 = mybir.dt.bfloat16
NP_BF16 = ml_dtypes.bfloat16

_prog_cache = {}


def make_constants():
    K = K_DIM
    hann = 0.5 * (1.0 - np.cos(2.0 * np.pi * np.arange(K) / K))
    # packed spectrum bins: j = 0..512 -> cos bin j; j = 513..1023 -> sin
    # bin (j - 512). 513 cos + 511 sin = exactly 1024 columns.
    j = np.arange(K)
    m_of_j = np.where(j <= 512, j, j - 512)
    is_sin = j > 512
    k = np.arange(K)[:, None].astype(np.float64)
    ang = 2 * np.pi * k * m_of_j[None, :] / K
    cs1 = hann[:, None] * np.where(is_sin[None, :], np.sin(ang), np.cos(ang))
    # DMA-friendly layout: [bin-chunk q][p][ktile t][j-in-chunk] so each
    # per-bin-chunk DMA reads one contiguous 256KB block.
    cs1 = cs1.reshape(8, 128, 8, 128)            # [t][p][q][jin]
    cs1 = np.ascontiguousarray(cs1.transpose(2, 1, 0, 3))  # [q][p][t][jin]
    cs1 = cs1.astype(np.float32).astype(NP_BF16)

    # irfft on pair-added bins: acov[d] = sum_{m<512} wt[m] P[m] cos(..)/K
    # + correction (chunk 4) for c512^2 riding in P[0]: (cos(pi d) - 1)/K
    # at partition 0 of the q=4 squares tile.
    wt = np.where(np.arange(512) == 0, 1.0, 2.0)
    d = np.arange(513)[None, :]
    m = np.arange(512)[:, None]
    c2a = wt[:, None] * np.cos(2 * np.pi * m * d / K) / K      # (512, 513)
    c2cor = np.zeros((128, 513))
    c2cor[0, :] = (np.cos(np.pi * d[0]) - 1.0) / K
    c2 = np.concatenate([c2a.reshape(4, 128, 513),
                         c2cor[None, :, :]], axis=0)           # (5, 128, 513)
    c2 = np.concatenate([c2, np.zeros((5, 128, 7))], axis=2)   # pad -> 520
    c2 = np.ascontiguousarray(c2.transpose(1, 0, 2))           # [p][t][d]
    c2 = c2.astype(np.float32).astype(NP_BF16)

    # acov[512] = (2/K) sum_p (-1)^p Q[p], with Q[p] = sum_t a_t[p] P[128t+p]
    # and a weight of 1/2 on P_add[0] only (its wt is 1, not 2; it already
    # contains c512^2, whose cos(pi*512) = 1 coefficient matches P[0]'s).
    # (-1)^m = (-1)^p since m = 128t + p. Output column 0 of an 8-wide tiny
    # matmul; columns 1..7 are zero pad.
    c2q = np.zeros((128, 8))
    p = np.arange(128)
    c2q[:, 0] = 2.0 * ((-1.0) ** p) / K
    c2q = c2q.astype(np.float32).astype(NP_BF16)
    w0 = np.ones((128, 1), dtype=np.float32)
    w0[0, 0] = 0.5

    ident = np.eye(128, dtype=np.float32).astype(NP_BF16)

    # gather start offsets (elements into flat [B_LOC*C*L] x) per tile row:
    # tile j: b = j//8, h = (j//4 % 2)*64 + (j%4)*16 + hh; row p = hh*8 + c
    gidx = np.zeros((16, 128), dtype=np.int32)
    for jt in range(16):
        b = jt // 8
        h0 = (jt // 4 % 2) * 64 + (jt % 4) * 16
        for hh in range(16):
            for c in range(C_DIM):
                gidx[jt, hh * 8 + c] = (
                    b * C_DIM * L_DIM + c * L_DIM + POS[h0 + hh]
                )
    gidx_t = np.ascontiguousarray(gidx.T)        # [p][tile]
    return {"cs1": cs1, "c2": c2, "c2q": c2q, "w0": w0, "ident": ident,
            "gidx": gidx_t}


def build_program():
    nc = bass.Bass("TRN2", target_bir_lowering=False, debug=False,
                   num_swdge_queues=4)
    x = nc.dram_tensor("x", [B_LOC, C_DIM, L_DIM], BF16,
                       kind="ExternalInput").ap()
    cs1 = nc.dram_tensor("cs1", [8, 128, 8, 128], BF16,
                         kind="ExternalInput").ap()
    c2 = nc.dram_tensor("c2", [128, 5, 520], BF16, kind="ExternalInput").ap()
    c2q = nc.dram_tensor("c2q", [128, 8], BF16, kind="ExternalInput").ap()
    w0 = nc.dram_tensor("w0", [128, 1], F32, kind="ExternalInput").ap()
    ident = nc.dram_tensor("ident", [128, 128], BF16,
                           kind="ExternalInput").ap()
    gidx = nc.dram_tensor("gidx", [128, 16], mybir.dt.int32,
                          kind="ExternalInput").ap()
    y = nc.dram_tensor(
        "y", [B_LOC, H_DIM, C_DIM, K_DIM], BF16, kind="ExternalOutput"
    ).ap()

    with tile.TileContext(nc) as tc:
        with (
            tc.tile_pool(name="singles", bufs=1) as singles,
            tc.tile_pool(name="gather", bufs=16) as gpool,
            tc.tile_pool(name="wt", bufs=2) as wtpool,
            tc.tile_pool(name="sq", bufs=2) as sqpool,
            tc.tile_pool(name="pa", bufs=2) as papool,
            tc.tile_pool(name="yy", bufs=4) as ypool,
            tc.tile_pool(name="small", bufs=48) as smallpool,
            tc.tile_pool(name="tp_ps", bufs=2, space="PSUM") as tp_ps_pool,
            tc.tile_pool(name="mm1_ps", bufs=2, space="PSUM") as mm1_ps_pool,
            tc.tile_pool(name="mm2_ps", bufs=2, space="PSUM") as mm2_ps_pool,
        ):
            # small control inputs first so gathers start immediately
            gidx_sb = singles.tile([128, 16], mybir.dt.int32)
            nc.sync.dma_start(out=gidx_sb, in_=gidx)
            id_sb = singles.tile([128, 128], BF16)
            nc.sync.dma_start(out=id_sb, in_=ident)
            x_flat = x.rearrange("b c l -> (b c) l")

            # all 16 gathers up front, spread over the 4 SWDGE queues
            gts = []
            for jt in range(16):
                gt = gpool.tile([128, K_DIM], BF16, tag="gt")
                gd = nc.gpsimd.indirect_dma_start(
                    out=gt[:],
                    out_offset=None,
                    in_=x_flat,
                    in_offset=bass.IndirectOffsetOnAxis(
                        ap=gidx_sb[:, jt : jt + 1], axis=1
                    ),
                )
                qi = jt % 4
                if qi:
                    gd.ins.queue = f"qPoolDynamic{qi}"
                gts.append(gt)

            # DFT matrices (bf16): cs1 per bin-chunk (contiguous 256KB each)
            cs1_sb = singles.tile([128, 8, K_DIM], BF16)
            for q in range(8):
                nc.sync.dma_start(
                    out=cs1_sb[:, :, 128 * q : 128 * (q + 1)], in_=cs1[q]
                )
            c2_sb = singles.tile([128, 5, 520], BF16)
            nc.sync.dma_start(out=c2_sb, in_=c2)
            c2q_sb = singles.tile([128, 8], BF16)
            nc.sync.dma_start(out=c2q_sb, in_=c2q)
            w0_sb = singles.tile([128, 1], F32)
            nc.sync.dma_start(out=w0_sb, in_=w0)

            # PE warmup: the gather triggers can't issue before the
            # gpsimd preamble (~10us), so the PE would idle cold and the
            # HAM clock-gate would hold it at half speed well into the
            # first groups. Burn idle time on dummy transposes (write-only
            # PSUM recycling, no readers) so the array is at full clock
            # when the real pipeline starts.
            for w in range(16):
                tpw = tp_ps_pool.tile([128, 512], BF16, tag="tp")
                for i in range(4):
                    nc.tensor.transpose(
                        tpw[:, 128 * i : 128 * (i + 1)],
                        id_sb, id_sb,
                    )

            inv2s = [None] * 16

            def tile_stats(jt):
                gt = gts[jt]
                fm = smallpool.tile([128, 1], F32, tag="fm")
                nc.vector.reduce_max(
                    out=fm, in_=gt,
                    axis=mybir.AxisListType.X,
                    apply_absolute_value=True,
                )
                inv = smallpool.tile([128, 1], F32, tag="inv")
                nc.vector.reciprocal(out=inv, in_=fm)
                inv2 = smallpool.tile([128, 1], F32, tag="inv2")
                nc.vector.tensor_mul(inv2, inv, inv)
                inv2s[jt] = inv2

            def mm1_q(gi, wt_t, sq, q, rows):
                # one spectrum tile [bin(128), row(rows)] + square to scratch
                mq = mm1_ps_pool.tile([128, 512], F32, tag="mm1")
                for t in range(8):
                    nc.tensor.matmul(
                        mq[:, 0:rows],
                        cs1_sb[:, t, 128 * q : 128 * (q + 1)],
                        wt_t[:, t, :],
                        start=(t == 0),
                        stop=(t == 7),
                    )
                nc.scalar.square(sq[:, q, :], mq[:, 0:rows])

            def mm2_rt(tile0, pa, sq, qv, rt):
                # acov[row, 0:513] for rows 128*rt..+127 of the group:
                # 4 chunks over the pair-added P + 1 correction chunk
                # (c512^2 rides in P[0]) + a tiny matmul over the
                # partition-folded Q for the acov[512] column; then expand
                # by symmetry: y[512+d] = acov[d], y[j] = acov[512-j],
                # scaled by 1/fmax^2.
                jt = tile0 + rt               # gather-tile index
                b = jt // 8
                hs = (jt // 4 % 2) * 64 + (jt % 4) * 16
                yp = mm2_ps_pool.tile([128, 520], F32, tag="mm2")
                r0 = 128 * rt
                nc.tensor.matmul(
                    yp[:, 512:520],
                    qv[:, r0 : r0 + 128],
                    c2q_sb,
                    start=True,
                    stop=True,
                )
                for t in range(4):
                    nc.tensor.matmul(
                        yp[:, 0:512],
                        pa[:, t, r0 : r0 + 128],
                        c2_sb[:, t, 0:512],
                        start=(t == 0),
                        stop=False,
                    )
                nc.tensor.matmul(
                    yp[:, 0:512],
                    sq[:, 4, r0 : r0 + 128],
                    c2_sb[:, 4, 0:512],
                    start=False,
                    stop=True,
                )
                ysb = ypool.tile([128, K_DIM], BF16, tag="y")
                inv2 = inv2s[jt]
                nc.vector.tensor_scalar_mul(
                    ysb[:, 512:1024], yp[:, 0:512], inv2
                )
                rev = bass.AP(
                    tensor=yp.tensor,
                    offset=yp.offset + 512,
                    ap=[list(yp.ap[0]), [-1, 512]],
                )
                nc.vector.tensor_scalar_mul(ysb[:, 0:512], rev, inv2)
                dst = y[b, hs : hs + 16].rearrange("h c n -> (h c) n")
                nc.sync.dma_start(out=dst, in_=ysb)

            # groups: three 512-row groups then two 256-row groups (the
            # smaller final groups thin out the kernel tail, where the
            # last group's mm2 cannot overlap any following work).
            # software pipeline: mm2(prev) rowtiles are interleaved
            # between transpose chunks and mm1 tiles of the current group
            # so y-copies drain on DVE while the PE keeps streaming.
            GROUPS = [(0, 4), (4, 4), (8, 4), (12, 2), (14, 2)]
            # per-group stats emission: front-load the reduce_max chains so
            # the last groups' DVE is free when their pair-adds must land
            # (a tile's gather is complete well before its stats slot)
            STATS = [(0, 1, 2, 3), (4, 5, 6, 7, 8, 9), (10, 11, 12, 13),
                     (14, 15), ()]
            prev = None            # (tile0, pa, sq, qv, ntiles)
            tail_yps = []

            def tail_mm2_chunk(pa, sq, t, yps):
                # last group: stream each irfft chunk as soon as its
                # pair-add lands, so only pa3/Q trail the final mm1
                for rt, yp in enumerate(yps):
                    r0 = 128 * rt
                    nc.tensor.matmul(
                        yp[:, 0:512],
                        pa[:, t, r0 : r0 + 128],
                        c2_sb[:, t, 0:512],
                        start=(t == 0),
                        stop=False,
                    )
                    if t == 0:
                        nc.tensor.matmul(
                            yp[:, 0:512],
                            sq[:, 4, r0 : r0 + 128],
                            c2_sb[:, 4, 0:512],
                            start=False,
                            stop=False,
                        )

            def tail_finish(tile0, qv, yps):
                for rt, yp in enumerate(yps):
                    jt = tile0 + rt
                    b = jt // 8
                    hs = (jt // 4 % 2) * 64 + (jt % 4) * 16
                    r0 = 128 * rt
                    nc.tensor.matmul(
                        yp[:, 512:520],
                        qv[:, r0 : r0 + 128],
                        c2q_sb,
                        start=True,
                        stop=True,
                    )
                    ysb = ypool.tile([128, K_DIM], BF16, tag="y")
                    inv2 = inv2s[jt]
                    nc.vector.tensor_scalar_mul(
                        ysb[:, 512:1024], yp[:, 0:512], inv2
                    )
                    rev = bass.AP(
                        tensor=yp.tensor,
                        offset=yp.offset + 512,
                        ap=[list(yp.ap[0]), [-1, 512]],
                    )
                    nc.vector.tensor_scalar_mul(ysb[:, 0:512], rev, inv2)
                    dst = y[b, hs : hs + 16].rearrange("h c n -> (h c) n")
                    nc.sync.dma_start(out=dst, in_=ysb)

            for gidx_i, (tile0, nt) in enumerate(GROUPS):
                last = gidx_i == len(GROUPS) - 1
                rows = 128 * nt
                wt_t = wtpool.tile([128, 8, 512], BF16, tag="wt")
                wt_t = wt_t[:, :, 0:rows]
                sq = sqpool.tile([128, 8, 512], BF16, tag="sq")
                sq = sq[:, :, 0:rows]
                pa = papool.tile([128, 4, 512], BF16, tag="pa")
                pa = pa[:, :, 0:rows]
                qv = papool.tile([128, 512], BF16, tag="qv")
                qv = qv[:, 0:rows]
                prt = 0 if prev is None else prev[4]
                for t in range(8):
                    tp = tp_ps_pool.tile([128, 512], BF16, tag="tp")
                    for i in range(nt):
                        nc.tensor.transpose(
                            tp[:, 128 * i : 128 * (i + 1)],
                            gts[tile0 + i][:, 128 * t : 128 * (t + 1)],
                            id_sb,
                        )
                    # alternate the PSUM->SBUF copies between ACT and DVE:
                    # two concurrent copy streams halve the transpose/copy
                    # ping-pong latency through the 2-buffer tp pool
                    if t % 2 == 0:
                        nc.scalar.copy(out=wt_t[:, t, :], in_=tp[:, 0:rows])
                    else:
                        nc.vector.tensor_copy(wt_t[:, t, :], tp[:, 0:rows])
                    if last:
                        if t == 1 and prt > 0:
                            mm2_rt(*prev[:4], 0)
                        if t == 3 and prt > 0:
                            mm2_rt(*prev[:4], 1)
                    elif t == 5 and prt > 0:
                        mm2_rt(*prev[:4], 0)
                # interleaved order (t, t+4) so each pair-add
                # P[m] = c_m^2 + s_m^2 issues right after its two squares;
                # per-tile stats (reduce_max chain) run here, off the
                # copy-bound transpose phase
                slist = STATS[gidx_i]
                for step, q in enumerate((0, 4, 1, 5, 2, 6, 3, 7)):
                    mm1_q(tile0, wt_t, sq, q, rows)
                    if step < len(slist):
                        tile_stats(slist[step])
                    if q >= 4:
                        t = q - 4
                        nc.vector.tensor_add(pa[:, t, :], sq[:, t, :],
                                             sq[:, q, :])
                        if t == 1:
                            nc.vector.scalar_tensor_tensor(
                                qv, pa[:, 0, :], w0_sb, pa[:, 1, :],
                                op0=mybir.AluOpType.mult,
                                op1=mybir.AluOpType.add,
                            )
                        elif t >= 2:
                            nc.vector.tensor_add(qv, qv, pa[:, t, :])
                    if step == 1 and prt > 2:
                        mm2_rt(*prev[:4], 1)
                    if step == 3 and prt > 2:
                        mm2_rt(*prev[:4], 2)
                    if step == 5 and prt > 2:
                        mm2_rt(*prev[:4], 3)
                    if step == 3 and not last and 0 < prt <= 2:
                        mm2_rt(*prev[:4], 1)
                    if last and step in (1, 3, 5, 7):
                        if step == 1:
                            tail_yps = [
                                mm2_ps_pool.tile(
                                    [128, 520], F32, tag="mm2",
                                    name=f"typ{_rt}",
                                )
                                for _rt in range(nt)
                            ]
                        tc_t = step // 2
                        tail_mm2_chunk(pa, sq, tc_t, tail_yps)
                prev = (tile0, pa, sq, qv, nt)
            tail_finish(GROUPS[-1][0], prev[3], tail_yps)
    return nc


def get_program():
    if "nc" not in _prog_cache:
        _prog_cache["nc"] = build_program()
        _prog_cache["consts"] = make_constants()
    return _prog_cache["nc"], _prog_cache["consts"]


def kernel(X, kernel_size=None, out_channels=None, _trace=False):
    X = np.ascontiguousarray(
        np.asarray(X, dtype=np.float32).astype(NP_BF16)
    )
    assert X.shape == (B_FULL, C_DIM, L_DIM)
    nc, consts = get_program()
    in_maps = []
    for c in range(N_CORES):
        m = {"x": X[c * B_LOC : (c + 1) * B_LOC]}
        m.update(consts)
        in_maps.append(m)
    res = run_bass_kernel_spmd(
        nc, in_maps, core_ids=list(range(N_CORES)), trace=_trace
    )
    out = np.concatenate(
        [np.asarray(r["y"]).astype(np.float32) for r in res.results], axis=0
    )
    if _trace:
        return out, res
    return out
